# revision 15
# baseline (speedup 1.0000x reference)
"""Trainium2 Bass kernel for nn_BatchDotPred: per-edge dot products of
gathered node features.

  edges: [E, 2] int, feat: [N, D] f32  ->  scores [E, 1] f32
  scores[e] = dot(feat[edges[e,0]], feat[edges[e,1]])

Strategy (8 NeuronCores, data parallel over edges):
  - E edges split into 8 contiguous shards of 250k, one per core.
  - The feat table is passed to every core as 4 chunk tensors of 25k rows
    each, so node indices local to a chunk fit the int16 index dtype of the
    InstDMAGatherAnt ucode (the fast SWDGE gather primitive; ~1.3 ns/row
    measured vs ~9 ns/row for a single SWDGE queue and far worse for generic
    indirect DMA).
  - Host-side, each core's edges are bucketed by (src_chunk, dst_chunk) -> 16
    buckets, each padded with dummy edges to a fixed capacity of CAP_TILES
    tiles x NI edges (so the single SPMD program has compile-time-constant
    shape; num_idxs_reg == NI always).
  - Per tile of NI=1024 edges: dma_gather the 1024 src rows (512 B each) from
    chunk a, dma_gather the 1024 dst rows from chunk b, DVE multiply, DVE
    segmented reduce -> [128, NI/128] scores.  Gathers cycle over the 4 SWDGE
    queues so descriptor generation runs on all four Q7 core pairs in
    parallel -- this is what reaches DMA line rate.
  - dma_gather writes row i of a tile to partition i%128, group i//128.
  - Scores accumulate in a persistent SBUF tile, written back with one DMA;
    host scatters them back to original edge order.
"""

import numpy as np

import concourse.bass as bass
import concourse.bacc as bacc
import concourse.tile as tile
import concourse.mybir as mybir
from concourse import bass_utils

N_CORES = 8
N_NODES = 100000
N_EDGES = 2000000
D = 128

N_CHUNKS = 4
CHUNK = N_NODES // N_CHUNKS          # 25000 rows per chunk tensor
N_BUCKETS = N_CHUNKS * N_CHUNKS      # 16

E_CORE = N_EDGES // N_CORES          # 250,000
BUCKET_CAP = 16384                   # edge capacity per bucket (mean 15625,
                                     # sd ~121 -> 6sd headroom)
SORT_BUCKETS = False
BUFS = 5


def set_tile_size(ni):
    """NI = edges per tile (= per dma_gather instruction)."""
    global NI, GROUPS, W, CAP_TILES, T_TOTAL, SLOTS
    NI = ni
    GROUPS = NI // 128
    W = NI // 16                     # idx columns per tile (wrapped int16)
    CAP_TILES = BUCKET_CAP // NI     # tiles per bucket
    T_TOTAL = N_BUCKETS * CAP_TILES  # tiles per core
    SLOTS = T_TOTAL * NI             # slots per core


set_tile_size(1024)

_programs = {}


def _build_program(reps=1, bufs=None):
    if bufs is None:
        bufs = BUFS
    nc = bacc.Bacc("TRN2", target_bir_lowering=False, debug=False,
                   num_devices=N_CORES, num_swdge_queues=4)
    chunk_aps = [
        nc.dram_tensor(f"feat{i}", [CHUNK, D], mybir.dt.int16,
                       kind="ExternalInput").ap()
        for i in range(N_CHUNKS)
    ]
    src_ap = nc.dram_tensor("src_idx", [128, T_TOTAL * W], mybir.dt.int16,
                            kind="ExternalInput").ap()
    dst_ap = nc.dram_tensor("dst_idx", [128, T_TOTAL * W], mybir.dt.int16,
                            kind="ExternalInput").ap()
    out_ap = nc.dram_tensor("scores", [128, T_TOTAL * GROUPS],
                            mybir.dt.float32, kind="ExternalOutput").ap()

    with tile.TileContext(nc) as tc:
        with tc.tile_pool(name="pool", bufs=bufs) as pool, \
             tc.tile_pool(name="persist", bufs=1) as persist:
            src_idx = persist.tile([128, T_TOTAL * W], mybir.dt.int16)
            dst_idx = persist.tile([128, T_TOTAL * W], mybir.dt.int16)
            scores = persist.tile([128, T_TOTAL * GROUPS], mybir.dt.float32)
            nc.sync.dma_start(src_idx[:], src_ap[:])
            nc.sync.dma_start(dst_idx[:], dst_ap[:])
            q = 0
            for _ in range(reps):
                for b in range(N_BUCKETS):
                    ca, cb = b // N_CHUNKS, b % N_CHUNKS
                    for t in range(CAP_TILES):
                        gt = b * CAP_TILES + t      # global tile id
                        isl = slice(gt * W, (gt + 1) * W)
                        s_tile = pool.tile([128, NI], mybir.dt.int16,
                                           tag="s")
                        d_tile = pool.tile([128, NI], mybir.dt.int16,
                                           tag="d")
                        nc.gpsimd.dma_gather(
                            out_ap=s_tile[:].rearrange("p (g d) -> p g d",
                                                       d=D),
                            in_ap=chunk_aps[ca][:],
                            idxs_ap=src_idx[:, isl],
                            num_idxs=NI, num_idxs_reg=NI, elem_size=D,
                            queue_num=q % 4)
                        q += 1
                        nc.gpsimd.dma_gather(
                            out_ap=d_tile[:].rearrange("p (g d) -> p g d",
                                                       d=D),
                            in_ap=chunk_aps[cb][:],
                            idxs_ap=dst_idx[:, isl],
                            num_idxs=NI, num_idxs_reg=NI, elem_size=D,
                            queue_num=q % 4)
                        q += 1
                        sb = s_tile[:].bitcast(mybir.dt.bfloat16)
                        db = d_tile[:].bitcast(mybir.dt.bfloat16)
                        nc.vector.tensor_mul(sb, sb, db)
                        nc.vector.tensor_reduce(
                            out=scores[:, gt * GROUPS:(gt + 1) * GROUPS],
                            in_=sb.rearrange("p (g d) -> p g d", d=D),
                            axis=mybir.AxisListType.X,
                            op=mybir.AluOpType.add)
            nc.sync.dma_start(out_ap[:], scores[:])

    nc.compile()
    return nc


def _get_program(reps=1):
    key = (reps, NI, BUFS)
    if key not in _programs:
        _programs[key] = _build_program(reps)
    return _programs[key]


def _wrap_idx(idx16: np.ndarray) -> np.ndarray:
    """[T_TOTAL*NI] int16 -> [128, T_TOTAL*W] wrapped+replicated layout."""
    # per tile: [NI] -> [W, 16] -> T to [16, W]; replicate to 128 partitions
    w = idx16.reshape(T_TOTAL, W, 16).transpose(0, 2, 1)   # [T, 16, W]
    w = np.tile(w, (1, 8, 1))                              # [T, 128, W]
    return np.ascontiguousarray(
        w.transpose(1, 0, 2).reshape(128, T_TOTAL * W))


def _pack_core(src: np.ndarray, dst: np.ndarray):
    """Bucket one core's edges; returns (src_wrapped, dst_wrapped, slot2edge).

    slot2edge: [SLOTS] int64, original edge position or -1 for padding.
    Raises if any bucket overflows CAP_TILES*NI (caller falls back).
    """
    ca = src // CHUNK
    cb = dst // CHUNK
    bucket = ca * N_CHUNKS + cb
    if SORT_BUCKETS:
        # secondary sort by src row for HBM row locality in the src gathers
        order = np.lexsort((src, bucket))
    else:
        order = np.argsort(bucket, kind="stable")
    counts = np.bincount(bucket, minlength=N_BUCKETS)
    if counts.max() > CAP_TILES * NI:
        raise OverflowError(f"bucket overflow: {counts.max()}")
    starts = np.zeros(N_BUCKETS, np.int64)
    starts[1:] = np.cumsum(counts)[:-1]

    slot2edge = np.full(SLOTS, -1, np.int64)
    src_slots = np.zeros(SLOTS, np.int16)
    dst_slots = np.zeros(SLOTS, np.int16)
    # dummy edges gather row 0 of the bucket's chunks (valid local index 0)
    for b in range(N_BUCKETS):
        seg = order[starts[b]:starts[b] + counts[b]]
        base = b * CAP_TILES * NI
        slot2edge[base:base + counts[b]] = seg
        src_slots[base:base + counts[b]] = (src[seg] % CHUNK).astype(np.int16)
        dst_slots[base:base + counts[b]] = (dst[seg] % CHUNK).astype(np.int16)
    return _wrap_idx(src_slots), _wrap_idx(dst_slots), slot2edge


def _unpack_scores(out: np.ndarray, slot2edge: np.ndarray) -> np.ndarray:
    """out: [128, T_TOTAL*GROUPS] -> [E_CORE] in original edge order."""
    # slot = t*NI + g*128 + p  ->  out[p, t*GROUPS + g]
    per_slot = out.reshape(128, T_TOTAL * GROUPS).T.reshape(
        T_TOTAL, GROUPS, 128).reshape(SLOTS)
    res = np.zeros(E_CORE, np.float32)
    valid = slot2edge >= 0
    res[slot2edge[valid]] = per_slot[valid]
    return res


def _f32_to_bf16_bits(x: np.ndarray) -> np.ndarray:
    """f32 -> bf16 bit pattern (round to nearest even), as int16."""
    u = np.ascontiguousarray(x, dtype=np.float32).view(np.uint32)
    bias = np.uint32(0x7FFF) + ((u >> np.uint32(16)) & np.uint32(1))
    return ((u + bias) >> np.uint32(16)).astype(np.uint16).view(np.int16)


def _make_in_maps(edges: np.ndarray, feat: np.ndarray):
    src = np.ascontiguousarray(edges[:, 0]).astype(np.int32, copy=False)
    dst = np.ascontiguousarray(edges[:, 1]).astype(np.int32, copy=False)
    chunks = {f"feat{i}": _f32_to_bf16_bits(feat[i * CHUNK:(i + 1) * CHUNK])
              for i in range(N_CHUNKS)}
    in_maps, slot_maps = [], []
    for c in range(N_CORES):
        s = src[c * E_CORE:(c + 1) * E_CORE]
        d = dst[c * E_CORE:(c + 1) * E_CORE]
        sw, dw, s2e = _pack_core(s, d)
        in_maps.append({**chunks, "src_idx": sw, "dst_idx": dw})
        slot_maps.append(s2e)
    return in_maps, slot_maps


def _run(edges: np.ndarray, feat: np.ndarray, trace: bool = False):
    edges = np.asarray(edges)
    feat = np.ascontiguousarray(np.asarray(feat, dtype=np.float32))
    assert edges.shape == (N_EDGES, 2) and feat.shape == (N_NODES, D)
    in_maps, slot_maps = _make_in_maps(edges, feat)
    nc = _get_program()
    res = bass_utils.run_bass_kernel_spmd(
        nc, in_maps, core_ids=list(range(N_CORES)), trace=trace)
    parts = [_unpack_scores(res.results[c]["scores"], slot_maps[c])
             for c in range(N_CORES)]
    return np.concatenate(parts).astype(np.float32)[:, None], res


def kernel(edges: np.ndarray, feat: np.ndarray) -> np.ndarray:
    out, _ = _run(edges, feat, trace=False)
    return out



# revision 16
# speedup vs baseline: 1.0211x; 1.0211x over previous
"""Trainium2 Bass kernel for nn_BatchDotPred: per-edge dot products of
gathered node features.

  edges: [E, 2] int, feat: [N, D] f32  ->  scores [E, 1] f32
  scores[e] = dot(feat[edges[e,0]], feat[edges[e,1]])

Strategy (8 NeuronCores, data parallel over edges):
  - E edges split into 8 contiguous shards of 250k, one per core.
  - The feat table is passed to every core as 4 chunk tensors of 25k rows
    each, so node indices local to a chunk fit the int16 index dtype of the
    InstDMAGatherAnt ucode (the fast SWDGE gather primitive; ~1.3 ns/row
    measured vs ~9 ns/row for a single SWDGE queue and far worse for generic
    indirect DMA).
  - Host-side, each core's edges are bucketed by (src_chunk, dst_chunk) -> 16
    buckets, each padded with dummy edges to a fixed capacity of CAP_TILES
    tiles x NI edges (so the single SPMD program has compile-time-constant
    shape; num_idxs_reg == NI always).
  - Per tile of NI=1024 edges: dma_gather the 1024 src rows (512 B each) from
    chunk a, dma_gather the 1024 dst rows from chunk b, DVE multiply, DVE
    segmented reduce -> [128, NI/128] scores.  Gathers cycle over the 4 SWDGE
    queues so descriptor generation runs on all four Q7 core pairs in
    parallel -- this is what reaches DMA line rate.
  - dma_gather writes row i of a tile to partition i%128, group i//128.
  - Scores accumulate in a persistent SBUF tile, written back with one DMA;
    host scatters them back to original edge order.
"""

import numpy as np

import concourse.bass as bass
import concourse.bacc as bacc
import concourse.tile as tile
import concourse.mybir as mybir
from concourse import bass_utils

N_CORES = 8
N_NODES = 100000
N_EDGES = 2000000
D = 128

N_CHUNKS = 4
CHUNK = N_NODES // N_CHUNKS          # 25000 rows per chunk tensor
N_BUCKETS = N_CHUNKS * N_CHUNKS      # 16

E_CORE = N_EDGES // N_CORES          # 250,000
BUCKET_CAP = 16384                   # edge capacity per bucket (mean 15625,
                                     # sd ~121 -> 6sd headroom)
SORT_BUCKETS = False
BUFS = 5


def set_tile_size(ni):
    """NI = edges per tile (= per dma_gather instruction)."""
    global NI, GROUPS, W, CAP_TILES, T_TOTAL, SLOTS
    NI = ni
    GROUPS = NI // 128
    W = NI // 16                     # idx columns per tile (wrapped int16)
    CAP_TILES = BUCKET_CAP // NI     # tiles per bucket
    T_TOTAL = N_BUCKETS * CAP_TILES  # tiles per core
    SLOTS = T_TOTAL * NI             # slots per core


set_tile_size(1024)

_programs = {}


def _build_program(reps=1, bufs=None):
    if bufs is None:
        bufs = BUFS
    nc = bacc.Bacc("TRN2", target_bir_lowering=False, debug=False,
                   num_devices=N_CORES, num_swdge_queues=4)
    chunk_aps = [
        nc.dram_tensor(f"feat{i}", [CHUNK, D], mybir.dt.float32,
                       kind="ExternalInput").ap()
        for i in range(N_CHUNKS)
    ]
    src_ap = nc.dram_tensor("src_idx", [128, T_TOTAL * W], mybir.dt.int16,
                            kind="ExternalInput").ap()
    dst_ap = nc.dram_tensor("dst_idx", [128, T_TOTAL * W], mybir.dt.int16,
                            kind="ExternalInput").ap()
    out_ap = nc.dram_tensor("scores", [128, T_TOTAL * GROUPS],
                            mybir.dt.float32, kind="ExternalOutput").ap()

    with tile.TileContext(nc) as tc:
        with tc.tile_pool(name="pool", bufs=bufs) as pool, \
             tc.tile_pool(name="persist", bufs=1) as persist:
            src_idx = persist.tile([128, T_TOTAL * W], mybir.dt.int16)
            dst_idx = persist.tile([128, T_TOTAL * W], mybir.dt.int16)
            scores = persist.tile([128, T_TOTAL * GROUPS], mybir.dt.float32)
            nc.sync.dma_start(src_idx[:], src_ap[:])
            nc.sync.dma_start(dst_idx[:], dst_ap[:])
            q = 0
            for _ in range(reps):
                for b in range(N_BUCKETS):
                    ca, cb = b // N_CHUNKS, b % N_CHUNKS
                    for t in range(CAP_TILES):
                        gt = b * CAP_TILES + t      # global tile id
                        isl = slice(gt * W, (gt + 1) * W)
                        s_tile = pool.tile([128, NI], mybir.dt.float32,
                                           tag="s")
                        d_tile = pool.tile([128, NI], mybir.dt.float32,
                                           tag="d")
                        nc.gpsimd.dma_gather(
                            out_ap=s_tile[:].rearrange("p (g d) -> p g d",
                                                       d=D),
                            in_ap=chunk_aps[ca][:],
                            idxs_ap=src_idx[:, isl],
                            num_idxs=NI, num_idxs_reg=NI, elem_size=D,
                            queue_num=q % 4)
                        q += 1
                        nc.gpsimd.dma_gather(
                            out_ap=d_tile[:].rearrange("p (g d) -> p g d",
                                                       d=D),
                            in_ap=chunk_aps[cb][:],
                            idxs_ap=dst_idx[:, isl],
                            num_idxs=NI, num_idxs_reg=NI, elem_size=D,
                            queue_num=q % 4)
                        q += 1
                        nc.vector.tensor_mul(s_tile[:], s_tile[:], d_tile[:])
                        nc.vector.tensor_reduce(
                            out=scores[:, gt * GROUPS:(gt + 1) * GROUPS],
                            in_=s_tile[:].rearrange("p (g d) -> p g d", d=D),
                            axis=mybir.AxisListType.X,
                            op=mybir.AluOpType.add)
            nc.sync.dma_start(out_ap[:], scores[:])

    nc.compile()
    return nc


def _get_program(reps=1):
    key = (reps, NI, BUFS)
    if key not in _programs:
        _programs[key] = _build_program(reps)
    return _programs[key]


def _wrap_idx(idx16: np.ndarray) -> np.ndarray:
    """[T_TOTAL*NI] int16 -> [128, T_TOTAL*W] wrapped+replicated layout."""
    # per tile: [NI] -> [W, 16] -> T to [16, W]; replicate to 128 partitions
    w = idx16.reshape(T_TOTAL, W, 16).transpose(0, 2, 1)   # [T, 16, W]
    w = np.tile(w, (1, 8, 1))                              # [T, 128, W]
    return np.ascontiguousarray(
        w.transpose(1, 0, 2).reshape(128, T_TOTAL * W))


def _pack_core(src: np.ndarray, dst: np.ndarray):
    """Bucket one core's edges; returns (src_wrapped, dst_wrapped, slot2edge).

    slot2edge: [SLOTS] int64, original edge position or -1 for padding.
    Raises if any bucket overflows CAP_TILES*NI (caller falls back).
    """
    ca = src // CHUNK
    cb = dst // CHUNK
    bucket = ca * N_CHUNKS + cb
    if SORT_BUCKETS:
        # secondary sort by src row for HBM row locality in the src gathers
        order = np.lexsort((src, bucket))
    else:
        order = np.argsort(bucket, kind="stable")
    counts = np.bincount(bucket, minlength=N_BUCKETS)
    if counts.max() > CAP_TILES * NI:
        raise OverflowError(f"bucket overflow: {counts.max()}")
    starts = np.zeros(N_BUCKETS, np.int64)
    starts[1:] = np.cumsum(counts)[:-1]

    slot2edge = np.full(SLOTS, -1, np.int64)
    src_slots = np.zeros(SLOTS, np.int16)
    dst_slots = np.zeros(SLOTS, np.int16)
    # dummy edges gather row 0 of the bucket's chunks (valid local index 0)
    for b in range(N_BUCKETS):
        seg = order[starts[b]:starts[b] + counts[b]]
        base = b * CAP_TILES * NI
        slot2edge[base:base + counts[b]] = seg
        src_slots[base:base + counts[b]] = (src[seg] % CHUNK).astype(np.int16)
        dst_slots[base:base + counts[b]] = (dst[seg] % CHUNK).astype(np.int16)
    return _wrap_idx(src_slots), _wrap_idx(dst_slots), slot2edge


def _unpack_scores(out: np.ndarray, slot2edge: np.ndarray) -> np.ndarray:
    """out: [128, T_TOTAL*GROUPS] -> [E_CORE] in original edge order."""
    # slot = t*NI + g*128 + p  ->  out[p, t*GROUPS + g]
    per_slot = out.reshape(128, T_TOTAL * GROUPS).T.reshape(
        T_TOTAL, GROUPS, 128).reshape(SLOTS)
    res = np.zeros(E_CORE, np.float32)
    valid = slot2edge >= 0
    res[slot2edge[valid]] = per_slot[valid]
    return res


def _make_in_maps(edges: np.ndarray, feat: np.ndarray):
    src = np.ascontiguousarray(edges[:, 0]).astype(np.int32, copy=False)
    dst = np.ascontiguousarray(edges[:, 1]).astype(np.int32, copy=False)
    chunks = {f"feat{i}": np.ascontiguousarray(feat[i * CHUNK:(i + 1) * CHUNK])
              for i in range(N_CHUNKS)}
    in_maps, slot_maps = [], []
    for c in range(N_CORES):
        s = src[c * E_CORE:(c + 1) * E_CORE]
        d = dst[c * E_CORE:(c + 1) * E_CORE]
        sw, dw, s2e = _pack_core(s, d)
        in_maps.append({**chunks, "src_idx": sw, "dst_idx": dw})
        slot_maps.append(s2e)
    return in_maps, slot_maps


def _run(edges: np.ndarray, feat: np.ndarray, trace: bool = False):
    edges = np.asarray(edges)
    feat = np.ascontiguousarray(np.asarray(feat, dtype=np.float32))
    assert edges.shape == (N_EDGES, 2) and feat.shape == (N_NODES, D)
    in_maps, slot_maps = _make_in_maps(edges, feat)
    nc = _get_program()
    res = bass_utils.run_bass_kernel_spmd(
        nc, in_maps, core_ids=list(range(N_CORES)), trace=trace)
    parts = [_unpack_scores(res.results[c]["scores"], slot_maps[c])
             for c in range(N_CORES)]
    return np.concatenate(parts).astype(np.float32)[:, None], res


def kernel(edges: np.ndarray, feat: np.ndarray) -> np.ndarray:
    out, _ = _run(edges, feat, trace=False)
    return out



# revision 17
# speedup vs baseline: 1.0342x; 1.0128x over previous
"""Trainium2 Bass kernel for nn_BatchDotPred: per-edge dot products of
gathered node features.

  edges: [E, 2] int, feat: [N, D] f32  ->  scores [E, 1] f32
  scores[e] = dot(feat[edges[e,0]], feat[edges[e,1]])

Strategy (8 NeuronCores, data parallel over edges):
  - E edges split into 8 contiguous shards of 250k, one per core.
  - The feat table is passed to every core as 4 chunk tensors of 25k rows
    each, so node indices local to a chunk fit the int16 index dtype of the
    InstDMAGatherAnt ucode (the fast SWDGE gather primitive; ~1.3 ns/row
    measured vs ~9 ns/row for a single SWDGE queue and far worse for generic
    indirect DMA).
  - Host-side, each core's edges are bucketed by (src_chunk, dst_chunk) -> 16
    buckets, each padded with dummy edges to a fixed capacity of CAP_TILES
    tiles x NI edges (so the single SPMD program has compile-time-constant
    shape; num_idxs_reg == NI always).
  - Per tile of NI=1024 edges: dma_gather the 1024 src rows (512 B each) from
    chunk a, dma_gather the 1024 dst rows from chunk b, DVE multiply, DVE
    segmented reduce -> [128, NI/128] scores.  Gathers cycle over the 4 SWDGE
    queues so descriptor generation runs on all four Q7 core pairs in
    parallel -- this is what reaches DMA line rate.
  - dma_gather writes row i of a tile to partition i%128, group i//128.
  - Scores accumulate in a persistent SBUF tile, written back with one DMA;
    host scatters them back to original edge order.
"""

import numpy as np

import concourse.bass as bass
import concourse.bacc as bacc
import concourse.tile as tile
import concourse.mybir as mybir
from concourse import bass_utils

N_CORES = 8
N_NODES = 100000
N_EDGES = 2000000
D = 128

N_CHUNKS = 4
CHUNK = N_NODES // N_CHUNKS          # 25000 rows per chunk tensor
N_BUCKETS = N_CHUNKS * N_CHUNKS      # 16

E_CORE = N_EDGES // N_CORES          # 250,000
BUCKET_CAP = 16384                   # edge capacity per bucket (mean 15625,
                                     # sd ~121 -> 6sd headroom)
SORT_BUCKETS = False
BUFS = 10


def set_tile_size(ni):
    """NI = edges per tile (= per dma_gather instruction)."""
    global NI, GROUPS, W, CAP_TILES, T_TOTAL, SLOTS
    NI = ni
    GROUPS = NI // 128
    W = NI // 16                     # idx columns per tile (wrapped int16)
    CAP_TILES = BUCKET_CAP // NI     # tiles per bucket
    T_TOTAL = N_BUCKETS * CAP_TILES  # tiles per core
    SLOTS = T_TOTAL * NI             # slots per core


set_tile_size(1024)

_programs = {}


def _build_program(reps=1, bufs=None):
    if bufs is None:
        bufs = BUFS
    nc = bacc.Bacc("TRN2", target_bir_lowering=False, debug=False,
                   num_devices=N_CORES, num_swdge_queues=4)
    chunk_aps = [
        nc.dram_tensor(f"feat{i}", [CHUNK, D], mybir.dt.float32,
                       kind="ExternalInput").ap()
        for i in range(N_CHUNKS)
    ]
    src_ap = nc.dram_tensor("src_idx", [128, T_TOTAL * W], mybir.dt.int16,
                            kind="ExternalInput").ap()
    dst_ap = nc.dram_tensor("dst_idx", [128, T_TOTAL * W], mybir.dt.int16,
                            kind="ExternalInput").ap()
    out_ap = nc.dram_tensor("scores", [128, T_TOTAL * GROUPS],
                            mybir.dt.float32, kind="ExternalOutput").ap()

    with tile.TileContext(nc) as tc:
        with tc.tile_pool(name="pool", bufs=bufs) as pool, \
             tc.tile_pool(name="persist", bufs=1) as persist:
            src_idx = persist.tile([128, T_TOTAL * W], mybir.dt.int16)
            dst_idx = persist.tile([128, T_TOTAL * W], mybir.dt.int16)
            scores = persist.tile([128, T_TOTAL * GROUPS], mybir.dt.float32)
            nc.sync.dma_start(src_idx[:], src_ap[:])
            nc.sync.dma_start(dst_idx[:], dst_ap[:])
            q = 0
            for _ in range(reps):
                for b in range(N_BUCKETS):
                    ca, cb = b // N_CHUNKS, b % N_CHUNKS
                    for t in range(CAP_TILES):
                        gt = b * CAP_TILES + t      # global tile id
                        isl = slice(gt * W, (gt + 1) * W)
                        s_tile = pool.tile([128, NI], mybir.dt.float32,
                                           tag="s")
                        d_tile = pool.tile([128, NI], mybir.dt.float32,
                                           tag="d")
                        nc.gpsimd.dma_gather(
                            out_ap=s_tile[:].rearrange("p (g d) -> p g d",
                                                       d=D),
                            in_ap=chunk_aps[ca][:],
                            idxs_ap=src_idx[:, isl],
                            num_idxs=NI, num_idxs_reg=NI, elem_size=D,
                            queue_num=q % 4)
                        q += 1
                        nc.gpsimd.dma_gather(
                            out_ap=d_tile[:].rearrange("p (g d) -> p g d",
                                                       d=D),
                            in_ap=chunk_aps[cb][:],
                            idxs_ap=dst_idx[:, isl],
                            num_idxs=NI, num_idxs_reg=NI, elem_size=D,
                            queue_num=q % 4)
                        q += 1
                        nc.vector.tensor_mul(s_tile[:], s_tile[:], d_tile[:])
                        nc.vector.tensor_reduce(
                            out=scores[:, gt * GROUPS:(gt + 1) * GROUPS],
                            in_=s_tile[:].rearrange("p (g d) -> p g d", d=D),
                            axis=mybir.AxisListType.X,
                            op=mybir.AluOpType.add)
            nc.sync.dma_start(out_ap[:], scores[:])

    nc.compile()
    return nc


def _get_program(reps=1):
    key = (reps, NI, BUFS)
    if key not in _programs:
        _programs[key] = _build_program(reps)
    return _programs[key]


def _wrap_idx(idx16: np.ndarray) -> np.ndarray:
    """[T_TOTAL*NI] int16 -> [128, T_TOTAL*W] wrapped+replicated layout."""
    # per tile: [NI] -> [W, 16] -> T to [16, W]; replicate to 128 partitions
    w = idx16.reshape(T_TOTAL, W, 16).transpose(0, 2, 1)   # [T, 16, W]
    w = np.tile(w, (1, 8, 1))                              # [T, 128, W]
    return np.ascontiguousarray(
        w.transpose(1, 0, 2).reshape(128, T_TOTAL * W))


def _pack_core(src: np.ndarray, dst: np.ndarray):
    """Bucket one core's edges; returns (src_wrapped, dst_wrapped, slot2edge).

    slot2edge: [SLOTS] int64, original edge position or -1 for padding.
    Raises if any bucket overflows CAP_TILES*NI (caller falls back).
    """
    ca = src // CHUNK
    cb = dst // CHUNK
    bucket = ca * N_CHUNKS + cb
    if SORT_BUCKETS:
        # secondary sort by src row for HBM row locality in the src gathers
        order = np.lexsort((src, bucket))
    else:
        order = np.argsort(bucket, kind="stable")
    counts = np.bincount(bucket, minlength=N_BUCKETS)
    if counts.max() > CAP_TILES * NI:
        raise OverflowError(f"bucket overflow: {counts.max()}")
    starts = np.zeros(N_BUCKETS, np.int64)
    starts[1:] = np.cumsum(counts)[:-1]

    slot2edge = np.full(SLOTS, -1, np.int64)
    src_slots = np.zeros(SLOTS, np.int16)
    dst_slots = np.zeros(SLOTS, np.int16)
    # dummy edges gather row 0 of the bucket's chunks (valid local index 0)
    for b in range(N_BUCKETS):
        seg = order[starts[b]:starts[b] + counts[b]]
        base = b * CAP_TILES * NI
        slot2edge[base:base + counts[b]] = seg
        src_slots[base:base + counts[b]] = (src[seg] % CHUNK).astype(np.int16)
        dst_slots[base:base + counts[b]] = (dst[seg] % CHUNK).astype(np.int16)
    return _wrap_idx(src_slots), _wrap_idx(dst_slots), slot2edge


def _unpack_scores(out: np.ndarray, slot2edge: np.ndarray) -> np.ndarray:
    """out: [128, T_TOTAL*GROUPS] -> [E_CORE] in original edge order."""
    # slot = t*NI + g*128 + p  ->  out[p, t*GROUPS + g]
    per_slot = out.reshape(128, T_TOTAL * GROUPS).T.reshape(
        T_TOTAL, GROUPS, 128).reshape(SLOTS)
    res = np.zeros(E_CORE, np.float32)
    valid = slot2edge >= 0
    res[slot2edge[valid]] = per_slot[valid]
    return res


def _make_in_maps(edges: np.ndarray, feat: np.ndarray):
    src = np.ascontiguousarray(edges[:, 0]).astype(np.int32, copy=False)
    dst = np.ascontiguousarray(edges[:, 1]).astype(np.int32, copy=False)
    chunks = {f"feat{i}": np.ascontiguousarray(feat[i * CHUNK:(i + 1) * CHUNK])
              for i in range(N_CHUNKS)}
    in_maps, slot_maps = [], []
    for c in range(N_CORES):
        s = src[c * E_CORE:(c + 1) * E_CORE]
        d = dst[c * E_CORE:(c + 1) * E_CORE]
        sw, dw, s2e = _pack_core(s, d)
        in_maps.append({**chunks, "src_idx": sw, "dst_idx": dw})
        slot_maps.append(s2e)
    return in_maps, slot_maps


def _run(edges: np.ndarray, feat: np.ndarray, trace: bool = False):
    edges = np.asarray(edges)
    feat = np.ascontiguousarray(np.asarray(feat, dtype=np.float32))
    assert edges.shape == (N_EDGES, 2) and feat.shape == (N_NODES, D)
    in_maps, slot_maps = _make_in_maps(edges, feat)
    nc = _get_program()
    res = bass_utils.run_bass_kernel_spmd(
        nc, in_maps, core_ids=list(range(N_CORES)), trace=trace)
    parts = [_unpack_scores(res.results[c]["scores"], slot_maps[c])
             for c in range(N_CORES)]
    return np.concatenate(parts).astype(np.float32)[:, None], res


def kernel(edges: np.ndarray, feat: np.ndarray) -> np.ndarray:
    out, _ = _run(edges, feat, trace=False)
    return out



# revision 18
# speedup vs baseline: 1.0491x; 1.0144x over previous
"""Trainium2 Bass kernel for nn_BatchDotPred: per-edge dot products of
gathered node features.

  edges: [E, 2] int, feat: [N, D] f32  ->  scores [E, 1] f32
  scores[e] = dot(feat[edges[e,0]], feat[edges[e,1]])

Strategy (8 NeuronCores, data parallel over edges):
  - E edges split into 8 contiguous shards of 250k, one per core.
  - The feat table is passed to every core as 4 chunk tensors of 25k rows
    each, so node indices local to a chunk fit the int16 index dtype of the
    InstDMAGatherAnt ucode (the fast SWDGE gather primitive; ~1.3 ns/row
    measured vs ~9 ns/row for a single SWDGE queue and far worse for generic
    indirect DMA).
  - Host-side, each core's edges are bucketed by (src_chunk, dst_chunk) -> 16
    buckets, each padded with dummy edges to a fixed capacity of CAP_TILES
    tiles x NI edges (so the single SPMD program has compile-time-constant
    shape; num_idxs_reg == NI always).
  - Per tile of NI=1024 edges: dma_gather the 1024 src rows (512 B each) from
    chunk a, dma_gather the 1024 dst rows from chunk b, DVE multiply, DVE
    segmented reduce -> [128, NI/128] scores.  Gathers cycle over the 4 SWDGE
    queues so descriptor generation runs on all four Q7 core pairs in
    parallel -- this is what reaches DMA line rate.
  - dma_gather writes row i of a tile to partition i%128, group i//128.
  - Scores accumulate in a persistent SBUF tile, written back with one DMA;
    host scatters them back to original edge order.
"""

import numpy as np

import concourse.bass as bass
import concourse.bacc as bacc
import concourse.tile as tile
import concourse.mybir as mybir
from concourse import bass_utils

N_CORES = 8
N_NODES = 100000
N_EDGES = 2000000
D = 128

N_CHUNKS = 4
CHUNK = N_NODES // N_CHUNKS          # 25000 rows per chunk tensor
N_BUCKETS = N_CHUNKS * N_CHUNKS      # 16

E_CORE = N_EDGES // N_CORES          # 250,000
BUCKET_CAP = 16384                   # edge capacity per bucket (mean 15625,
                                     # sd ~121 -> 6sd headroom)
SORT_BUCKETS = False
BUFS = 10


def set_tile_size(ni):
    """NI = edges per tile (= per dma_gather instruction)."""
    global NI, GROUPS, W, CAP_TILES, T_TOTAL, SLOTS
    NI = ni
    GROUPS = NI // 128
    W = NI // 16                     # idx columns per tile (wrapped int16)
    CAP_TILES = BUCKET_CAP // NI     # tiles per bucket
    T_TOTAL = N_BUCKETS * CAP_TILES  # tiles per core
    SLOTS = T_TOTAL * NI             # slots per core


set_tile_size(1024)

_programs = {}


def _build_program(reps=1, bufs=None):
    if bufs is None:
        bufs = BUFS
    nc = bacc.Bacc("TRN2", target_bir_lowering=False, debug=False,
                   num_devices=N_CORES, num_swdge_queues=4)
    chunk_aps = [
        nc.dram_tensor(f"feat{i}", [CHUNK, D], mybir.dt.float32,
                       kind="ExternalInput").ap()
        for i in range(N_CHUNKS)
    ]
    src_ap = nc.dram_tensor("src_idx", [128, T_TOTAL * W], mybir.dt.int16,
                            kind="ExternalInput").ap()
    dst_ap = nc.dram_tensor("dst_idx", [128, T_TOTAL * W], mybir.dt.int16,
                            kind="ExternalInput").ap()
    out_ap = nc.dram_tensor("scores", [128, T_TOTAL * GROUPS],
                            mybir.dt.float32, kind="ExternalOutput").ap()

    with tile.TileContext(nc) as tc:
        with tc.tile_pool(name="pool", bufs=bufs) as pool, \
             tc.tile_pool(name="persist", bufs=1) as persist:
            src_idx = persist.tile([128, T_TOTAL * W], mybir.dt.int16)
            dst_idx = persist.tile([128, T_TOTAL * W], mybir.dt.int16)
            scores = persist.tile([128, T_TOTAL * GROUPS], mybir.dt.float32)
            npart = 4
            part = T_TOTAL * W // npart
            for i in range(npart):
                sl = slice(i * part, (i + 1) * part)
                nc.sync.dma_start(src_idx[:, sl], src_ap[:, sl])
                nc.scalar.dma_start(dst_idx[:, sl], dst_ap[:, sl])
            q = 0
            for _ in range(reps):
                for b in range(N_BUCKETS):
                    ca, cb = b // N_CHUNKS, b % N_CHUNKS
                    for t in range(CAP_TILES):
                        gt = b * CAP_TILES + t      # global tile id
                        isl = slice(gt * W, (gt + 1) * W)
                        s_tile = pool.tile([128, NI], mybir.dt.float32,
                                           tag="s")
                        d_tile = pool.tile([128, NI], mybir.dt.float32,
                                           tag="d")
                        nc.gpsimd.dma_gather(
                            out_ap=s_tile[:].rearrange("p (g d) -> p g d",
                                                       d=D),
                            in_ap=chunk_aps[ca][:],
                            idxs_ap=src_idx[:, isl],
                            num_idxs=NI, num_idxs_reg=NI, elem_size=D,
                            queue_num=q % 4)
                        q += 1
                        nc.gpsimd.dma_gather(
                            out_ap=d_tile[:].rearrange("p (g d) -> p g d",
                                                       d=D),
                            in_ap=chunk_aps[cb][:],
                            idxs_ap=dst_idx[:, isl],
                            num_idxs=NI, num_idxs_reg=NI, elem_size=D,
                            queue_num=q % 4)
                        q += 1
                        nc.vector.tensor_mul(s_tile[:], s_tile[:], d_tile[:])
                        nc.vector.tensor_reduce(
                            out=scores[:, gt * GROUPS:(gt + 1) * GROUPS],
                            in_=s_tile[:].rearrange("p (g d) -> p g d", d=D),
                            axis=mybir.AxisListType.X,
                            op=mybir.AluOpType.add)
            nc.sync.dma_start(out_ap[:], scores[:])

    nc.compile()
    return nc


def _get_program(reps=1):
    key = (reps, NI, BUFS)
    if key not in _programs:
        _programs[key] = _build_program(reps)
    return _programs[key]


def _wrap_idx(idx16: np.ndarray) -> np.ndarray:
    """[T_TOTAL*NI] int16 -> [128, T_TOTAL*W] wrapped+replicated layout."""
    # per tile: [NI] -> [W, 16] -> T to [16, W]; replicate to 128 partitions
    w = idx16.reshape(T_TOTAL, W, 16).transpose(0, 2, 1)   # [T, 16, W]
    w = np.tile(w, (1, 8, 1))                              # [T, 128, W]
    return np.ascontiguousarray(
        w.transpose(1, 0, 2).reshape(128, T_TOTAL * W))


def _pack_core(src: np.ndarray, dst: np.ndarray):
    """Bucket one core's edges; returns (src_wrapped, dst_wrapped, slot2edge).

    slot2edge: [SLOTS] int64, original edge position or -1 for padding.
    Raises if any bucket overflows CAP_TILES*NI (caller falls back).
    """
    ca = src // CHUNK
    cb = dst // CHUNK
    bucket = ca * N_CHUNKS + cb
    if SORT_BUCKETS:
        # secondary sort by src row for HBM row locality in the src gathers
        order = np.lexsort((src, bucket))
    else:
        order = np.argsort(bucket, kind="stable")
    counts = np.bincount(bucket, minlength=N_BUCKETS)
    if counts.max() > CAP_TILES * NI:
        raise OverflowError(f"bucket overflow: {counts.max()}")
    starts = np.zeros(N_BUCKETS, np.int64)
    starts[1:] = np.cumsum(counts)[:-1]

    slot2edge = np.full(SLOTS, -1, np.int64)
    src_slots = np.zeros(SLOTS, np.int16)
    dst_slots = np.zeros(SLOTS, np.int16)
    # dummy edges gather row 0 of the bucket's chunks (valid local index 0)
    for b in range(N_BUCKETS):
        seg = order[starts[b]:starts[b] + counts[b]]
        base = b * CAP_TILES * NI
        slot2edge[base:base + counts[b]] = seg
        src_slots[base:base + counts[b]] = (src[seg] % CHUNK).astype(np.int16)
        dst_slots[base:base + counts[b]] = (dst[seg] % CHUNK).astype(np.int16)
    return _wrap_idx(src_slots), _wrap_idx(dst_slots), slot2edge


def _unpack_scores(out: np.ndarray, slot2edge: np.ndarray) -> np.ndarray:
    """out: [128, T_TOTAL*GROUPS] -> [E_CORE] in original edge order."""
    # slot = t*NI + g*128 + p  ->  out[p, t*GROUPS + g]
    per_slot = out.reshape(128, T_TOTAL * GROUPS).T.reshape(
        T_TOTAL, GROUPS, 128).reshape(SLOTS)
    res = np.zeros(E_CORE, np.float32)
    valid = slot2edge >= 0
    res[slot2edge[valid]] = per_slot[valid]
    return res


def _make_in_maps(edges: np.ndarray, feat: np.ndarray):
    src = np.ascontiguousarray(edges[:, 0]).astype(np.int32, copy=False)
    dst = np.ascontiguousarray(edges[:, 1]).astype(np.int32, copy=False)
    chunks = {f"feat{i}": np.ascontiguousarray(feat[i * CHUNK:(i + 1) * CHUNK])
              for i in range(N_CHUNKS)}
    in_maps, slot_maps = [], []
    for c in range(N_CORES):
        s = src[c * E_CORE:(c + 1) * E_CORE]
        d = dst[c * E_CORE:(c + 1) * E_CORE]
        sw, dw, s2e = _pack_core(s, d)
        in_maps.append({**chunks, "src_idx": sw, "dst_idx": dw})
        slot_maps.append(s2e)
    return in_maps, slot_maps


def _run(edges: np.ndarray, feat: np.ndarray, trace: bool = False):
    edges = np.asarray(edges)
    feat = np.ascontiguousarray(np.asarray(feat, dtype=np.float32))
    assert edges.shape == (N_EDGES, 2) and feat.shape == (N_NODES, D)
    in_maps, slot_maps = _make_in_maps(edges, feat)
    nc = _get_program()
    res = bass_utils.run_bass_kernel_spmd(
        nc, in_maps, core_ids=list(range(N_CORES)), trace=trace)
    parts = [_unpack_scores(res.results[c]["scores"], slot_maps[c])
             for c in range(N_CORES)]
    return np.concatenate(parts).astype(np.float32)[:, None], res


def kernel(edges: np.ndarray, feat: np.ndarray) -> np.ndarray:
    out, _ = _run(edges, feat, trace=False)
    return out



# revision 19
# speedup vs baseline: 1.1062x; 1.0544x over previous
"""Trainium2 Bass kernel for nn_BatchDotPred: per-edge dot products of
gathered node features.

  edges: [E, 2] int, feat: [N, D] f32  ->  scores [E, 1] f32
  scores[e] = dot(feat[edges[e,0]], feat[edges[e,1]])

Strategy (8 NeuronCores, data parallel over edges):
  - E edges split into 8 contiguous shards of 250k, one per core.
  - The feat table is passed to every core as 4 chunk tensors of 25k rows
    each, so node indices local to a chunk fit the int16 index dtype of the
    InstDMAGatherAnt ucode (the fast SWDGE gather primitive; ~1.3 ns/row
    measured vs ~9 ns/row for a single SWDGE queue and far worse for generic
    indirect DMA).
  - Host-side, each core's edges are bucketed by (src_chunk, dst_chunk) -> 16
    buckets, each padded with dummy edges to a fixed capacity of CAP_TILES
    tiles x NI edges (so the single SPMD program has compile-time-constant
    shape; num_idxs_reg == NI always).
  - Per tile of NI=1024 edges: dma_gather the 1024 src rows (512 B each) from
    chunk a, dma_gather the 1024 dst rows from chunk b, DVE multiply, DVE
    segmented reduce -> [128, NI/128] scores.  Gathers cycle over the 4 SWDGE
    queues so descriptor generation runs on all four Q7 core pairs in
    parallel -- this is what reaches DMA line rate.
  - dma_gather writes row i of a tile to partition i%128, group i//128.
  - Scores accumulate in a persistent SBUF tile, written back with one DMA;
    host scatters them back to original edge order.
"""

import numpy as np

import concourse.bass as bass
import concourse.bacc as bacc
import concourse.tile as tile
import concourse.mybir as mybir
from concourse import bass_utils

N_CORES = 8
N_NODES = 100000
N_EDGES = 2000000
D = 128

N_CHUNKS = 4
CHUNK = N_NODES // N_CHUNKS          # 25000 rows per chunk tensor
N_BUCKETS = N_CHUNKS * N_CHUNKS      # 16

E_CORE = N_EDGES // N_CORES          # 250,000
BUCKET_CAP = 16384                   # edge capacity per bucket (mean 15625,
                                     # sd ~121 -> 6sd headroom)
SORT_BUCKETS = False
BUFS = 14


def set_tile_size(ni):
    """NI = edges per tile (= per dma_gather instruction)."""
    global NI, GROUPS, W, CAP_TILES, T_TOTAL, SLOTS
    NI = ni
    GROUPS = NI // 128
    W = NI // 16                     # idx columns per tile (wrapped int16)
    CAP_TILES = BUCKET_CAP // NI     # tiles per bucket
    T_TOTAL = N_BUCKETS * CAP_TILES  # tiles per core
    SLOTS = T_TOTAL * NI             # slots per core


set_tile_size(1024)

_programs = {}


def _build_program(reps=1, bufs=None):
    if bufs is None:
        bufs = BUFS
    nc = bacc.Bacc("TRN2", target_bir_lowering=False, debug=False,
                   num_devices=N_CORES, num_swdge_queues=4)
    chunk_aps = [
        nc.dram_tensor(f"feat{i}", [CHUNK, D], mybir.dt.float32,
                       kind="ExternalInput").ap()
        for i in range(N_CHUNKS)
    ]
    src_ap = nc.dram_tensor("src_idx", [128, T_TOTAL * W], mybir.dt.int16,
                            kind="ExternalInput").ap()
    dst_ap = nc.dram_tensor("dst_idx", [128, T_TOTAL * W], mybir.dt.int16,
                            kind="ExternalInput").ap()
    out_ap = nc.dram_tensor("scores", [128, T_TOTAL * GROUPS],
                            mybir.dt.float32, kind="ExternalOutput").ap()

    with tile.TileContext(nc) as tc:
        with tc.tile_pool(name="pool", bufs=bufs) as pool, \
             tc.tile_pool(name="persist", bufs=1) as persist:
            src_idx = persist.tile([128, T_TOTAL * W], mybir.dt.int16)
            dst_idx = persist.tile([128, T_TOTAL * W], mybir.dt.int16)
            scores = persist.tile([128, T_TOTAL * GROUPS], mybir.dt.float32)
            npart = 8
            part = T_TOTAL * W // npart
            for i in range(npart):
                sl = slice(i * part, (i + 1) * part)
                nc.sync.dma_start(src_idx[:, sl], src_ap[:, sl])
                nc.scalar.dma_start(dst_idx[:, sl], dst_ap[:, sl])
            q = 0
            for _ in range(reps):
                for b in range(N_BUCKETS):
                    ca, cb = b // N_CHUNKS, b % N_CHUNKS
                    for t in range(CAP_TILES):
                        gt = b * CAP_TILES + t      # global tile id
                        isl = slice(gt * W, (gt + 1) * W)
                        s_tile = pool.tile([128, NI], mybir.dt.float32,
                                           tag="s")
                        d_tile = pool.tile([128, NI], mybir.dt.float32,
                                           tag="d")
                        nc.gpsimd.dma_gather(
                            out_ap=s_tile[:].rearrange("p (g d) -> p g d",
                                                       d=D),
                            in_ap=chunk_aps[ca][:],
                            idxs_ap=src_idx[:, isl],
                            num_idxs=NI, num_idxs_reg=NI, elem_size=D,
                            queue_num=q % 4)
                        q += 1
                        nc.gpsimd.dma_gather(
                            out_ap=d_tile[:].rearrange("p (g d) -> p g d",
                                                       d=D),
                            in_ap=chunk_aps[cb][:],
                            idxs_ap=dst_idx[:, isl],
                            num_idxs=NI, num_idxs_reg=NI, elem_size=D,
                            queue_num=q % 4)
                        q += 1
                        nc.vector.tensor_mul(s_tile[:], s_tile[:], d_tile[:])
                        nc.vector.tensor_reduce(
                            out=scores[:, gt * GROUPS:(gt + 1) * GROUPS],
                            in_=s_tile[:].rearrange("p (g d) -> p g d", d=D),
                            axis=mybir.AxisListType.X,
                            op=mybir.AluOpType.add)
            nc.sync.dma_start(out_ap[:], scores[:])

    nc.compile()
    return nc


def _get_program(reps=1):
    key = (reps, NI, BUFS)
    if key not in _programs:
        _programs[key] = _build_program(reps)
    return _programs[key]


def _wrap_idx(idx16: np.ndarray) -> np.ndarray:
    """[T_TOTAL*NI] int16 -> [128, T_TOTAL*W] wrapped+replicated layout."""
    # per tile: [NI] -> [W, 16] -> T to [16, W]; replicate to 128 partitions
    w = idx16.reshape(T_TOTAL, W, 16).transpose(0, 2, 1)   # [T, 16, W]
    w = np.tile(w, (1, 8, 1))                              # [T, 128, W]
    return np.ascontiguousarray(
        w.transpose(1, 0, 2).reshape(128, T_TOTAL * W))


def _pack_core(src: np.ndarray, dst: np.ndarray):
    """Bucket one core's edges; returns (src_wrapped, dst_wrapped, slot2edge).

    slot2edge: [SLOTS] int64, original edge position or -1 for padding.
    Raises if any bucket overflows CAP_TILES*NI (caller falls back).
    """
    ca = src // CHUNK
    cb = dst // CHUNK
    bucket = ca * N_CHUNKS + cb
    if SORT_BUCKETS:
        # secondary sort by src row for HBM row locality in the src gathers
        order = np.lexsort((src, bucket))
    else:
        order = np.argsort(bucket, kind="stable")
    counts = np.bincount(bucket, minlength=N_BUCKETS)
    if counts.max() > CAP_TILES * NI:
        raise OverflowError(f"bucket overflow: {counts.max()}")
    starts = np.zeros(N_BUCKETS, np.int64)
    starts[1:] = np.cumsum(counts)[:-1]

    slot2edge = np.full(SLOTS, -1, np.int64)
    src_slots = np.zeros(SLOTS, np.int16)
    dst_slots = np.zeros(SLOTS, np.int16)
    # dummy edges gather row 0 of the bucket's chunks (valid local index 0)
    for b in range(N_BUCKETS):
        seg = order[starts[b]:starts[b] + counts[b]]
        base = b * CAP_TILES * NI
        slot2edge[base:base + counts[b]] = seg
        src_slots[base:base + counts[b]] = (src[seg] % CHUNK).astype(np.int16)
        dst_slots[base:base + counts[b]] = (dst[seg] % CHUNK).astype(np.int16)
    return _wrap_idx(src_slots), _wrap_idx(dst_slots), slot2edge


def _unpack_scores(out: np.ndarray, slot2edge: np.ndarray) -> np.ndarray:
    """out: [128, T_TOTAL*GROUPS] -> [E_CORE] in original edge order."""
    # slot = t*NI + g*128 + p  ->  out[p, t*GROUPS + g]
    per_slot = out.reshape(128, T_TOTAL * GROUPS).T.reshape(
        T_TOTAL, GROUPS, 128).reshape(SLOTS)
    res = np.zeros(E_CORE, np.float32)
    valid = slot2edge >= 0
    res[slot2edge[valid]] = per_slot[valid]
    return res


def _make_in_maps(edges: np.ndarray, feat: np.ndarray):
    src = np.ascontiguousarray(edges[:, 0]).astype(np.int32, copy=False)
    dst = np.ascontiguousarray(edges[:, 1]).astype(np.int32, copy=False)
    chunks = {f"feat{i}": np.ascontiguousarray(feat[i * CHUNK:(i + 1) * CHUNK])
              for i in range(N_CHUNKS)}
    in_maps, slot_maps = [], []
    for c in range(N_CORES):
        s = src[c * E_CORE:(c + 1) * E_CORE]
        d = dst[c * E_CORE:(c + 1) * E_CORE]
        sw, dw, s2e = _pack_core(s, d)
        in_maps.append({**chunks, "src_idx": sw, "dst_idx": dw})
        slot_maps.append(s2e)
    return in_maps, slot_maps


def _run(edges: np.ndarray, feat: np.ndarray, trace: bool = False):
    edges = np.asarray(edges)
    feat = np.ascontiguousarray(np.asarray(feat, dtype=np.float32))
    assert edges.shape == (N_EDGES, 2) and feat.shape == (N_NODES, D)
    in_maps, slot_maps = _make_in_maps(edges, feat)
    nc = _get_program()
    res = bass_utils.run_bass_kernel_spmd(
        nc, in_maps, core_ids=list(range(N_CORES)), trace=trace)
    parts = [_unpack_scores(res.results[c]["scores"], slot_maps[c])
             for c in range(N_CORES)]
    return np.concatenate(parts).astype(np.float32)[:, None], res


def kernel(edges: np.ndarray, feat: np.ndarray) -> np.ndarray:
    out, _ = _run(edges, feat, trace=False)
    return out

